# revision 4
# baseline (speedup 1.0000x reference)
"""Butterfly rotation (10 stages, DIM=1024) on 8 Trainium2 NeuronCores.

Math: each row x[n, :] undergoes 10 butterfly rotation stages; the whole
transform is linear.  Stages 0..8 (strides 1..256) only mix elements within
each 512-wide half, so their composite is block-diagonal with two dense
512x512 blocks H0, H1 (precomputed on host from `angles`).  Stage 9
(stride 512) pairs element k with k+512 and is applied on-chip as a cheap
per-element rotation.

Device layout (per core, rows sharded 8192/core):
  - host pre-transposes each core's shard to dim-major tiles
    xin[g, p, c*512 + r] = x[g*512 + r, c*128 + p]   (g: 16 row-groups,
    c: 8 dim-chunks of 128, p: partition = dim-within-chunk, r: row)
  - PE: per group and output chunk cg (0..3):
        ps_lo[m, r] = sum_ci H0[ci*128+k, cg*128+m] * x[row, ci*128+k]
        ps_hi[m, r] = same with H1 / chunks 4..7
    i.e. 32 fp32 matmuls [K=128, M=128, N=512] per group.
  - stage 9 fused with PSUM eviction:
        y[k]     = cos_k * y1[k] - sin_k * y1[k+512]
        y[k+512] = sin_k * y1[k] + cos_k * y1[k+512]
    with per-partition scalars (ACT does the ps_hi pre-scales, DVE the
    fused multiply-adds), written straight into the dim-major output tile.
  - host inverse-permutes the output tiles.
"""

import os
import sys

sys.path.insert(0, "/opt/trn_rl_repo")

# run_bass_kernel_spmd would try to import the (absent) axon NTFF hook if
# BASS_TRACE is set in the environment.
os.environ["BASS_NEVER_TRACE"] = "1"

import numpy as np

DIM = 1024
STAGES = 10
N_CORES = 8
ROWS_PER_CORE = 8192
GROUP_ROWS = 512
N_GROUPS = ROWS_PER_CORE // GROUP_ROWS  # 16
F32 = None  # set after bass import


def _stage_idx(dim, stage):
    stride = 2**stage
    idx_i = np.arange(dim).reshape(-1, 2 * stride)[:, :stride].ravel()
    idx_j = idx_i + stride
    return idx_i, idx_j


def _butterfly_apply(v, angles, stages):
    """Apply butterfly stages to rows of v (float64, in place) and return v."""
    for s in stages:
        idx_i, idx_j = _stage_idx(v.shape[1], s)
        c = np.cos(angles[s].astype(np.float64))
        sn = np.sin(angles[s].astype(np.float64))
        vi = v[:, idx_i].copy()
        vj = v[:, idx_j].copy()
        v[:, idx_i] = c * vi - sn * vj
        v[:, idx_j] = sn * vi + c * vj
    return v


def _host_tables(angles):
    """Precompute the stationary matmul blocks and stage-9 trig tables."""
    ma = _butterfly_apply(np.eye(DIM, dtype=np.float64), angles, range(STAGES - 1))
    h0 = ma[:512, :512]
    h1 = ma[512:, 512:]
    assert abs(ma[:512, 512:]).max() == 0.0 and abs(ma[512:, :512]).max() == 0.0

    # wts[k, b, m], b = cg*4 + ci: lhsT block for output chunk cg, input chunk ci
    wts = np.empty((128, 32, 128), dtype=np.float32)
    for cg in range(8):
        h = h0 if cg < 4 else h1
        jl = (cg % 4) * 128
        for ci in range(4):
            wts[:, cg * 4 + ci, :] = h[ci * 128 : (ci + 1) * 128, jl : jl + 128].astype(
                np.float32
            )

    th = angles[STAGES - 1].astype(np.float64)
    cos9 = np.cos(th)
    sin9 = np.sin(th)
    # trig[m, 0:4] = cos, [m, 4:8] = sin, [m, 8:12] = -sin  (per chunk cg)
    trig = np.empty((128, 12), dtype=np.float32)
    for cg in range(4):
        sl = slice(cg * 128, (cg + 1) * 128)
        trig[:, cg] = cos9[sl]
        trig[:, 4 + cg] = sin9[sl]
        trig[:, 8 + cg] = -sin9[sl]
    return wts, trig


def _pack_x(x_core, n_groups=N_GROUPS):
    # [G*512, 1024] -> [G, 128, 4096] with xin[g, p, c*512+r] = x[g*512+r, c*128+p]
    g = x_core.reshape(n_groups, GROUP_ROWS, 8, 128)
    return np.ascontiguousarray(g.transpose(0, 3, 2, 1).reshape(n_groups, 128, 4096))


def _unpack_y(y_packed, n_groups=N_GROUPS):
    # inverse of _pack_x
    g = y_packed.reshape(n_groups, 128, 8, GROUP_ROWS)
    return np.ascontiguousarray(
        g.transpose(0, 3, 2, 1).reshape(n_groups * GROUP_ROWS, DIM)
    )


def _patch_tile_drain():
    """Workaround: this walrus build cannot encode semaphore waits on a
    sequencer Drain/NoOp with >1 wait ("Too many sync wait commands").
    Re-emit the TileContext tail waits as one nop per semaphore."""
    from concourse import mybir, tile
    from concourse.vector_clock import ScopedClock

    if getattr(tile.TileContext, "_drain_patched", False):
        return

    def _drain_and_barrier(self, tick_clock, wait_clock):
        nop_inst = self.nc.sync.nop(nofuse=True)
        wait_clock.add_sem_waits(
            nop_inst.ins, ScopedClock({None: tick_clock.global_clock})
        )
        si = nop_inst.ins.sync_info
        if si is not None and si.on_wait and len(si.on_wait) > 1:
            extra = si.on_wait[1:]
            si.on_wait = si.on_wait[:1]
            for w in extra:
                extra_nop = self.nc.sync.nop(nofuse=True)
                esi = extra_nop.ins.sync_info
                if esi is None:
                    extra_nop.ins.sync_info = mybir.SyncInfo(on_wait=[w], on_update=[])
                else:
                    esi.on_wait = list(esi.on_wait or []) + [w]
        self.nc.sync.drain()
        self.nc.all_engine_barrier()
        assert self.sems is not None
        popped = self.nc._tile_sem_poison_stack.pop()
        assert popped is self._sem_poison
        self.nc.clear_and_free_semaphores(list(self.sems.allocated().values()))
        self.nc.all_engine_barrier()

    tile.TileContext._drain_and_barrier = _drain_and_barrier
    tile.TileContext._drain_patched = True


def _split_multi_waits(nc, limit=1):
    """This walrus build encodes at most `limit` semaphore wait(s) per
    instruction ("Too many sync wait commands").  Hoist excess waits onto
    same-engine NoOps inserted immediately before the instruction."""
    from concourse import mybir

    counter = [0]

    def fresh_nop(engine, waits):
        counter[0] += 1
        nop = mybir.InstNoOp(
            name=f"waitsplit-{counter[0]}",
            engine=engine,
            ins=[],
            outs=[],
            bass_nofuse=True,
            sync_info=mybir.SyncInfo(on_wait=list(waits), on_update=[]),
        )
        nc.register_instruction(nop, overwrite=True)
        return nop

    for fn in nc.m.functions:
        for bb in fn.blocks:
            changed = False
            new = []
            for inst in bb.instructions:
                si = getattr(inst, "sync_info", None)
                if si is not None and si.on_wait and len(si.on_wait) > limit:
                    extra = si.on_wait[: len(si.on_wait) - limit]
                    si.on_wait = si.on_wait[len(si.on_wait) - limit :]
                    for k in range(0, len(extra), limit):
                        new.append(fresh_nop(inst.engine, extra[k : k + limit]))
                    changed = True
                new.append(inst)
            if changed:
                bb.instructions = new


def build_bass(n_groups=N_GROUPS):
    """Build the Bass module for one core processing n_groups row-groups."""
    _patch_tile_drain()
    from concourse import bass, mybir, tile

    f32 = mybir.dt.float32
    nc = bass.Bass("TRN2", target_bir_lowering=False, debug=False)
    xin = nc.dram_tensor("xin", [n_groups, 128, 4096], f32, kind="ExternalInput")
    wts = nc.dram_tensor("wts", [128, 32, 128], f32, kind="ExternalInput")
    trig = nc.dram_tensor("trig", [128, 12], f32, kind="ExternalInput")
    yout = nc.dram_tensor("yout", [n_groups, 128, 4096], f32, kind="ExternalOutput")

    mult = mybir.AluOpType.mult
    add = mybir.AluOpType.add
    copy_fn = mybir.ActivationFunctionType.Copy

    with tile.TileContext(nc) as tc:
        with (
            tc.tile_pool(name="wp", bufs=1) as wp,
            tc.tile_pool(name="xp", bufs=3) as xp,
            tc.tile_pool(name="yp", bufs=3) as yp,
            tc.tile_pool(name="tp", bufs=4) as tp,
            tc.tile_pool(name="ps", bufs=4, space="PSUM") as psp,
        ):
            wt = wp.tile([128, 32, 128], f32)
            nc.sync.dma_start(wt[:], wts.ap()[:])
            tg = wp.tile([128, 12], f32)
            nc.sync.dma_start(tg[:], trig.ap()[:])

            for g in range(n_groups):
                xt = xp.tile([128, 4096], f32)
                nc.sync.dma_start(xt[:], xin.ap()[g])
                yt = yp.tile([128, 4096], f32)
                for cg in range(4):
                    ps_lo = psp.tile([128, 512], f32)
                    ps_hi = psp.tile([128, 512], f32)
                    for ci in range(4):
                        nc.tensor.matmul(
                            ps_lo[:],
                            wt[:, cg * 4 + ci, :],
                            xt[:, ci * 512 : (ci + 1) * 512],
                            start=(ci == 0),
                            stop=(ci == 3),
                        )
                    for ci in range(4):
                        nc.tensor.matmul(
                            ps_hi[:],
                            wt[:, (cg + 4) * 4 + ci, :],
                            xt[:, (4 + ci) * 512 : (5 + ci) * 512],
                            start=(ci == 0),
                            stop=(ci == 3),
                        )
                    # stage 9: y_lo = cos*lo - sin*hi ; y_hi = sin*lo + cos*hi
                    t1 = tp.tile([128, 512], f32)
                    nc.scalar.activation(
                        t1[:], ps_hi[:], copy_fn, scale=tg[:, 8 + cg : 9 + cg]
                    )
                    nc.vector.scalar_tensor_tensor(
                        yt[:, cg * 512 : (cg + 1) * 512],
                        ps_lo[:],
                        tg[:, cg : cg + 1],
                        t1[:],
                        mult,
                        add,
                    )
                    t2 = tp.tile([128, 512], f32)
                    nc.scalar.activation(
                        t2[:], ps_hi[:], copy_fn, scale=tg[:, cg : cg + 1]
                    )
                    nc.vector.scalar_tensor_tensor(
                        yt[:, (4 + cg) * 512 : (5 + cg) * 512],
                        ps_lo[:],
                        tg[:, 4 + cg : 5 + cg],
                        t2[:],
                        mult,
                        add,
                    )
                nc.sync.dma_start(yout.ap()[g], yt[:])
    _split_multi_waits(nc)
    return nc


_CACHE = {}


def _get_nc(n_groups=N_GROUPS):
    if n_groups not in _CACHE:
        _CACHE[n_groups] = build_bass(n_groups)
    return _CACHE[n_groups]


def make_in_maps(x, angles):
    """Pack full inputs into per-core in_maps (list of dicts)."""
    x = np.asarray(x, dtype=np.float32)
    angles = np.asarray(angles, dtype=np.float32)
    wts, trig = _host_tables(angles)
    flat = x.reshape(-1, DIM)
    in_maps = []
    for k in range(N_CORES):
        shard = flat[k * ROWS_PER_CORE : (k + 1) * ROWS_PER_CORE]
        in_maps.append({"xin": _pack_x(shard), "wts": wts, "trig": trig})
    return in_maps


def kernel(x, angles):
    from concourse.bass_utils import run_bass_kernel_spmd

    x = np.asarray(x)
    orig_shape = x.shape
    in_maps = make_in_maps(x, angles)
    nc = _get_nc()
    res = run_bass_kernel_spmd(nc, in_maps, core_ids=list(range(N_CORES)))
    parts = [_unpack_y(res.results[k]["yout"]) for k in range(N_CORES)]
    out = np.concatenate(parts, axis=0).reshape(orig_shape)
    return out.astype(np.float32)


# revision 6
# speedup vs baseline: 19.5604x; 19.5604x over previous
"""Butterfly rotation (10 stages, DIM=1024) on 8 Trainium2 NeuronCores.

Math: each row x[n, :] undergoes 10 butterfly rotation stages; the whole
transform is linear.  Stages 0..8 (strides 1..256) only mix elements within
each 512-wide half, so their composite is block-diagonal with two dense
512x512 blocks H0, H1 (precomputed on host from `angles`).  Stage 9
(stride 512) pairs element k with k+512 and is applied on-chip as a cheap
per-element rotation.

Device layout (per core, rows sharded 8192/core):
  - host pre-transposes each core's shard to dim-major tiles
    xin[g, p, c*512 + r] = x[g*512 + r, c*128 + p]   (g: 16 row-groups,
    c: 8 dim-chunks of 128, p: partition = dim-within-chunk, r: row)
  - PE: per group and output chunk cg (0..3):
        ps_lo[m, r] = sum_ci H0[ci*128+k, cg*128+m] * x[row, ci*128+k]
        ps_hi[m, r] = same with H1 / chunks 4..7
    i.e. 32 fp32 matmuls [K=128, M=128, N=512] per group.
  - stage 9 fused with PSUM eviction:
        y[k]     = cos_k * y1[k] - sin_k * y1[k+512]
        y[k+512] = sin_k * y1[k] + cos_k * y1[k+512]
    with per-partition scalars (ACT does the ps_hi pre-scales, DVE the
    fused multiply-adds), written straight into the dim-major output tile.
  - host inverse-permutes the output tiles.
"""

import os
import sys

sys.path.insert(0, "/opt/trn_rl_repo")

# run_bass_kernel_spmd would try to import the (absent) axon NTFF hook if
# BASS_TRACE is set in the environment.
os.environ["BASS_NEVER_TRACE"] = "1"

import numpy as np

DIM = 1024
STAGES = 10
N_CORES = 8
ROWS_PER_CORE = 8192
GROUP_ROWS = 512
N_GROUPS = ROWS_PER_CORE // GROUP_ROWS  # 16
F32 = None  # set after bass import


def _stage_idx(dim, stage):
    stride = 2**stage
    idx_i = np.arange(dim).reshape(-1, 2 * stride)[:, :stride].ravel()
    idx_j = idx_i + stride
    return idx_i, idx_j


def _butterfly_apply(v, angles, stages):
    """Apply butterfly stages to rows of v (float64, in place) and return v."""
    for s in stages:
        idx_i, idx_j = _stage_idx(v.shape[1], s)
        c = np.cos(angles[s].astype(np.float64))
        sn = np.sin(angles[s].astype(np.float64))
        vi = v[:, idx_i].copy()
        vj = v[:, idx_j].copy()
        v[:, idx_i] = c * vi - sn * vj
        v[:, idx_j] = sn * vi + c * vj
    return v


def _host_tables(angles):
    """Precompute the stationary matmul blocks and stage-9 trig tables."""
    ma = _butterfly_apply(np.eye(DIM, dtype=np.float64), angles, range(STAGES - 1))
    h0 = ma[:512, :512]
    h1 = ma[512:, 512:]
    assert abs(ma[:512, 512:]).max() == 0.0 and abs(ma[512:, :512]).max() == 0.0

    # wts[k, b, m], b = cg*4 + ci: lhsT block for output chunk cg, input chunk ci
    wts = np.empty((128, 32, 128), dtype=np.float32)
    for cg in range(8):
        h = h0 if cg < 4 else h1
        jl = (cg % 4) * 128
        for ci in range(4):
            wts[:, cg * 4 + ci, :] = h[ci * 128 : (ci + 1) * 128, jl : jl + 128].astype(
                np.float32
            )

    th = angles[STAGES - 1].astype(np.float64)
    cos9 = np.cos(th)
    sin9 = np.sin(th)
    # trig[m, 0:4] = cos, [m, 4:8] = sin, [m, 8:12] = -sin  (per chunk cg)
    trig = np.empty((128, 12), dtype=np.float32)
    for cg in range(4):
        sl = slice(cg * 128, (cg + 1) * 128)
        trig[:, cg] = cos9[sl]
        trig[:, 4 + cg] = sin9[sl]
        trig[:, 8 + cg] = -sin9[sl]
    return wts, trig


def _pack_x(x_core, n_groups=N_GROUPS):
    # [G*512, 1024] -> [G, 128, 4096] with xin[g, p, c*512+r] = x[g*512+r, c*128+p]
    g = x_core.reshape(n_groups, GROUP_ROWS, 8, 128)
    return np.ascontiguousarray(g.transpose(0, 3, 2, 1).reshape(n_groups, 128, 4096))


def _unpack_y(y_packed, n_groups=N_GROUPS):
    # inverse of _pack_x
    g = y_packed.reshape(n_groups, 128, 8, GROUP_ROWS)
    return np.ascontiguousarray(
        g.transpose(0, 3, 2, 1).reshape(n_groups * GROUP_ROWS, DIM)
    )


def _patch_tile_drain():
    """Workaround: this walrus build cannot encode semaphore waits on a
    sequencer Drain/NoOp with >1 wait ("Too many sync wait commands").
    Re-emit the TileContext tail waits as one nop per semaphore."""
    from concourse import mybir, tile
    from concourse.vector_clock import ScopedClock

    if getattr(tile.TileContext, "_drain_patched", False):
        return

    def _drain_and_barrier(self, tick_clock, wait_clock):
        nop_inst = self.nc.sync.nop(nofuse=True)
        wait_clock.add_sem_waits(
            nop_inst.ins, ScopedClock({None: tick_clock.global_clock})
        )
        si = nop_inst.ins.sync_info
        if si is not None and si.on_wait and len(si.on_wait) > 1:
            extra = si.on_wait[1:]
            si.on_wait = si.on_wait[:1]
            for w in extra:
                extra_nop = self.nc.sync.nop(nofuse=True)
                esi = extra_nop.ins.sync_info
                if esi is None:
                    extra_nop.ins.sync_info = mybir.SyncInfo(on_wait=[w], on_update=[])
                else:
                    esi.on_wait = list(esi.on_wait or []) + [w]
        self.nc.sync.drain()
        self.nc.all_engine_barrier()
        assert self.sems is not None
        popped = self.nc._tile_sem_poison_stack.pop()
        assert popped is self._sem_poison
        self.nc.clear_and_free_semaphores(list(self.sems.allocated().values()))
        self.nc.all_engine_barrier()

    tile.TileContext._drain_and_barrier = _drain_and_barrier
    tile.TileContext._drain_patched = True


def _split_multi_waits(nc, limit=1):
    """This walrus build encodes at most `limit` semaphore wait(s) per
    instruction ("Too many sync wait commands").  Hoist excess waits onto
    same-engine NoOps inserted immediately before the instruction."""
    from concourse import mybir

    counter = [0]

    def fresh_nop(engine, waits):
        counter[0] += 1
        nop = mybir.InstNoOp(
            name=f"waitsplit-{counter[0]}",
            engine=engine,
            ins=[],
            outs=[],
            bass_nofuse=True,
            sync_info=mybir.SyncInfo(on_wait=list(waits), on_update=[]),
        )
        nc.register_instruction(nop, overwrite=True)
        return nop

    for fn in nc.m.functions:
        for bb in fn.blocks:
            changed = False
            new = []
            for inst in bb.instructions:
                si = getattr(inst, "sync_info", None)
                if si is not None and si.on_wait and len(si.on_wait) > limit:
                    extra = si.on_wait[: len(si.on_wait) - limit]
                    si.on_wait = si.on_wait[len(si.on_wait) - limit :]
                    for k in range(0, len(extra), limit):
                        new.append(fresh_nop(inst.engine, extra[k : k + limit]))
                    changed = True
                new.append(inst)
            if changed:
                bb.instructions = new


def build_bass(n_groups=N_GROUPS, reps=1):
    """Build the Bass module for one core processing n_groups row-groups.
    reps>1 repeats the whole pipeline in-NEFF (for timing calibration)."""
    _patch_tile_drain()
    from concourse import bass, mybir, tile

    f32 = mybir.dt.float32
    nc = bass.Bass("TRN2", target_bir_lowering=False, debug=False)
    xin = nc.dram_tensor("xin", [n_groups, 128, 4096], f32, kind="ExternalInput")
    wts = nc.dram_tensor("wts", [128, 32, 128], f32, kind="ExternalInput")
    trig = nc.dram_tensor("trig", [128, 12], f32, kind="ExternalInput")
    yout = nc.dram_tensor("yout", [n_groups, 128, 4096], f32, kind="ExternalOutput")

    mult = mybir.AluOpType.mult
    add = mybir.AluOpType.add
    copy_fn = mybir.ActivationFunctionType.Copy

    with tile.TileContext(nc) as tc:
        with (
            tc.tile_pool(name="wp", bufs=1) as wp,
            tc.tile_pool(name="xp", bufs=3) as xp,
            tc.tile_pool(name="yp", bufs=3) as yp,
            tc.tile_pool(name="tp", bufs=4) as tp,
            tc.tile_pool(name="ps", bufs=4, space="PSUM") as psp,
        ):
            wt = wp.tile([128, 32, 128], f32)
            nc.sync.dma_start(wt[:], wts.ap()[:])
            tg = wp.tile([128, 12], f32)
            nc.sync.dma_start(tg[:], trig.ap()[:])

            for g in [g for _ in range(reps) for g in range(n_groups)]:
                xt = xp.tile([128, 4096], f32)
                nc.sync.dma_start(xt[:], xin.ap()[g])
                yt = yp.tile([128, 4096], f32)
                for cg in range(4):
                    ps_lo = psp.tile([128, 512], f32)
                    ps_hi = psp.tile([128, 512], f32)
                    for ci in range(4):
                        nc.tensor.matmul(
                            ps_lo[:],
                            wt[:, cg * 4 + ci, :],
                            xt[:, ci * 512 : (ci + 1) * 512],
                            start=(ci == 0),
                            stop=(ci == 3),
                        )
                    for ci in range(4):
                        nc.tensor.matmul(
                            ps_hi[:],
                            wt[:, (cg + 4) * 4 + ci, :],
                            xt[:, (4 + ci) * 512 : (5 + ci) * 512],
                            start=(ci == 0),
                            stop=(ci == 3),
                        )
                    # stage 9: y_lo = cos*lo - sin*hi ; y_hi = sin*lo + cos*hi
                    t1 = tp.tile([128, 512], f32)
                    nc.scalar.activation(
                        t1[:], ps_hi[:], copy_fn, scale=tg[:, 8 + cg : 9 + cg]
                    )
                    nc.vector.scalar_tensor_tensor(
                        yt[:, cg * 512 : (cg + 1) * 512],
                        ps_lo[:],
                        tg[:, cg : cg + 1],
                        t1[:],
                        mult,
                        add,
                    )
                    t2 = tp.tile([128, 512], f32)
                    nc.scalar.activation(
                        t2[:], ps_hi[:], copy_fn, scale=tg[:, cg : cg + 1]
                    )
                    nc.vector.scalar_tensor_tensor(
                        yt[:, (4 + cg) * 512 : (5 + cg) * 512],
                        ps_lo[:],
                        tg[:, 4 + cg : 5 + cg],
                        t2[:],
                        mult,
                        add,
                    )
                nc.sync.dma_start(yout.ap()[g], yt[:])
    _split_multi_waits(nc)
    return nc


_CACHE = {}


def _get_nc(n_groups=N_GROUPS):
    if n_groups not in _CACHE:
        _CACHE[n_groups] = build_bass(n_groups)
    return _CACHE[n_groups]


def make_in_maps(x, angles):
    """Pack full inputs into per-core in_maps (list of dicts)."""
    x = np.asarray(x, dtype=np.float32)
    angles = np.asarray(angles, dtype=np.float32)
    wts, trig = _host_tables(angles)
    flat = x.reshape(-1, DIM)
    in_maps = []
    for k in range(N_CORES):
        shard = flat[k * ROWS_PER_CORE : (k + 1) * ROWS_PER_CORE]
        in_maps.append({"xin": _pack_x(shard), "wts": wts, "trig": trig})
    return in_maps


def kernel(x, angles):
    from concourse.bass_utils import run_bass_kernel_spmd

    x = np.asarray(x)
    orig_shape = x.shape
    in_maps = make_in_maps(x, angles)
    nc = _get_nc()
    res = run_bass_kernel_spmd(nc, in_maps, core_ids=list(range(N_CORES)))
    parts = [_unpack_y(res.results[k]["yout"]) for k in range(N_CORES)]
    out = np.concatenate(parts, axis=0).reshape(orig_shape)
    return out.astype(np.float32)


# revision 14
# speedup vs baseline: 43.5755x; 2.2277x over previous
"""Butterfly rotation (10 stages, DIM=1024) on 8 Trainium2 NeuronCores.

Math: each row x[n, :] undergoes 10 butterfly rotation stages; the whole
transform is linear.  Stages 0..7 (strides 1..128) only mix elements within
256-wide blocks, so their composite is block-diagonal with four dense
256x256 blocks (precomputed on host from `angles`).  Stages 8 and 9
(strides 256/512) pair whole 128-dim chunks and are applied on-chip as
per-element rotations with per-partition cos/sin scalars.

Device layout (per core, rows sharded 8192/core; pure data parallelism):
  - host pre-transposes each core's shard to dim-major tiles
    xin[g, p, c*512 + r] = x[g*512 + r, c*128 + p]   (g: 16 row-groups,
    c: 8 dim-chunks of 128, p: partition = dim-within-chunk, r: row)
  - PE: per group, 16 fp32 matmuls [K=128, M=128, N=512] (2 accumulating
    per output chunk) compute the stages-0..7 result in PSUM.
  - stage 8 (chunk pairs (0,2),(1,3),(4,6),(5,7)) evicts PSUM -> SBUF:
    ACT does the cross-term pre-scales, DVE the fused multiply-adds
    (scalar_tensor_tensor).  stage 9 (pairs (c, c+4)) repeats this
    SBUF -> SBUF into the output tile; each finished 1 MiB slice is
    DMA-stored immediately (stores on the ACT HWDGE ring, loads on SP).
  - host inverse-permutes the output tiles.

Empirical note: fp32 [128,128,512] matmuls measure ~1.05 us each here
(cold-clock + fp32 stream rate), so minimizing matmul count (16/group via
the 0..7 split, vs 32 for 0..8 or 64 with stage-9 folded into weights)
is what gets the kernel under the DMA roofline.
"""

import os
import sys

sys.path.insert(0, "/opt/trn_rl_repo")

# run_bass_kernel_spmd would try to import the (absent) axon NTFF hook if
# BASS_TRACE is set in the environment.
os.environ["BASS_NEVER_TRACE"] = "1"

import numpy as np

DIM = 1024
STAGES = 10
N_CORES = 8
ROWS_PER_CORE = 8192
GROUP_ROWS = 512
N_GROUPS = ROWS_PER_CORE // GROUP_ROWS  # 16
F32 = None  # set after bass import


def _stage_idx(dim, stage):
    stride = 2**stage
    idx_i = np.arange(dim).reshape(-1, 2 * stride)[:, :stride].ravel()
    idx_j = idx_i + stride
    return idx_i, idx_j


def _butterfly_apply(v, angles, stages):
    """Apply butterfly stages to rows of v (float64, in place) and return v."""
    for s in stages:
        idx_i, idx_j = _stage_idx(v.shape[1], s)
        c = np.cos(angles[s].astype(np.float64))
        sn = np.sin(angles[s].astype(np.float64))
        vi = v[:, idx_i].copy()
        vj = v[:, idx_j].copy()
        v[:, idx_i] = c * vi - sn * vj
        v[:, idx_j] = sn * vi + c * vj
    return v


def _host_tables(angles):
    """Stages 0..7 (strides 1..128) mix only within 256-wide blocks: their
    composite is block-diagonal with four dense 256x256 blocks B_q.  Stages 8
    and 9 are applied on-chip as per-element rotations between dim-chunks.

    wts[k, b, m], b = c*2 + t: lhsT block for output chunk c (0..7), input
    chunk ci = 2*(c//2) + t; block = B_{c//2}[t*128:(t+1)*128, (c%2)*128:...].

    trig[m, :]: stage-8 tables per chunk-pair (pairs (0,2),(1,3),(4,6),(5,7),
    first-chunk angle offsets [0,128,256,384]), then stage-9 per cg:
      cols 0:4   cos8[pidx], 4:8 sin8, 8:12 -sin8,
      cols 12:16 cos9[cg],  16:20 sin9, 20:24 -sin9.
    """
    mb = _butterfly_apply(np.eye(DIM, dtype=np.float64), angles, range(STAGES - 2))
    wts = np.empty((128, 16, 128), dtype=np.float32)
    for c in range(8):
        q = c // 2
        blk = mb[q * 256 : (q + 1) * 256, q * 256 : (q + 1) * 256]
        jl = (c % 2) * 128
        for t in range(2):
            wts[:, c * 2 + t, :] = blk[t * 128 : (t + 1) * 128, jl : jl + 128].astype(
                np.float32
            )
    # off-block-diagonal must vanish for stages 0..7
    mask = np.ones((DIM, DIM), dtype=bool)
    for q in range(4):
        mask[q * 256 : (q + 1) * 256, q * 256 : (q + 1) * 256] = False
    assert abs(mb[mask]).max() == 0.0

    th8 = angles[8].astype(np.float64)
    th9 = angles[9].astype(np.float64)
    trig = np.empty((128, 24), dtype=np.float32)
    for pidx, off in enumerate([0, 128, 256, 384]):
        sl = slice(off, off + 128)
        trig[:, pidx] = np.cos(th8[sl])
        trig[:, 4 + pidx] = np.sin(th8[sl])
        trig[:, 8 + pidx] = -np.sin(th8[sl])
    for cg in range(4):
        sl = slice(cg * 128, (cg + 1) * 128)
        trig[:, 12 + cg] = np.cos(th9[sl])
        trig[:, 16 + cg] = np.sin(th9[sl])
        trig[:, 20 + cg] = -np.sin(th9[sl])
    return wts, trig


def _pack_x(x_core, n_groups=N_GROUPS):
    # [G*512, 1024] -> [G, 128, 4096] with xin[g, p, c*512+r] = x[g*512+r, c*128+p]
    g = x_core.reshape(n_groups, GROUP_ROWS, 8, 128)
    return np.ascontiguousarray(g.transpose(0, 3, 2, 1).reshape(n_groups, 128, 4096))


def _unpack_y(y_packed, n_groups=N_GROUPS):
    # yout[g, p, cg*1024 + h*512 + r] = y[g*512 + r, (h*4 + cg)*128 + p]
    g = y_packed.reshape(n_groups, 128, 4, 2, GROUP_ROWS)
    return np.ascontiguousarray(
        g.transpose(0, 4, 3, 2, 1).reshape(n_groups * GROUP_ROWS, DIM)
    )


def _patch_tile_drain():
    """Workaround: this walrus build cannot encode semaphore waits on a
    sequencer Drain/NoOp with >1 wait ("Too many sync wait commands").
    Re-emit the TileContext tail waits as one nop per semaphore."""
    from concourse import mybir, tile
    from concourse.vector_clock import ScopedClock

    if getattr(tile.TileContext, "_drain_patched", False):
        return

    def _drain_and_barrier(self, tick_clock, wait_clock):
        nop_inst = self.nc.sync.nop(nofuse=True)
        wait_clock.add_sem_waits(
            nop_inst.ins, ScopedClock({None: tick_clock.global_clock})
        )
        si = nop_inst.ins.sync_info
        if si is not None and si.on_wait and len(si.on_wait) > 1:
            extra = si.on_wait[1:]
            si.on_wait = si.on_wait[:1]
            for w in extra:
                extra_nop = self.nc.sync.nop(nofuse=True)
                esi = extra_nop.ins.sync_info
                if esi is None:
                    extra_nop.ins.sync_info = mybir.SyncInfo(on_wait=[w], on_update=[])
                else:
                    esi.on_wait = list(esi.on_wait or []) + [w]
        self.nc.sync.drain()
        self.nc.all_engine_barrier()
        assert self.sems is not None
        popped = self.nc._tile_sem_poison_stack.pop()
        assert popped is self._sem_poison
        self.nc.clear_and_free_semaphores(list(self.sems.allocated().values()))
        self.nc.all_engine_barrier()

    tile.TileContext._drain_and_barrier = _drain_and_barrier
    tile.TileContext._drain_patched = True


def _split_multi_waits(nc, limit=1):
    """This walrus build encodes at most `limit` semaphore wait(s) per
    instruction ("Too many sync wait commands").  Hoist excess waits onto
    same-engine NoOps inserted immediately before the instruction."""
    from concourse import mybir

    counter = [0]

    def fresh_nop(engine, waits):
        counter[0] += 1
        nop = mybir.InstNoOp(
            name=f"waitsplit-{counter[0]}",
            engine=engine,
            ins=[],
            outs=[],
            bass_nofuse=True,
            sync_info=mybir.SyncInfo(on_wait=list(waits), on_update=[]),
        )
        nc.register_instruction(nop, overwrite=True)
        return nop

    for fn in nc.m.functions:
        for bb in fn.blocks:
            changed = False
            new = []
            for inst in bb.instructions:
                si = getattr(inst, "sync_info", None)
                if si is not None and si.on_wait and len(si.on_wait) > limit:
                    extra = si.on_wait[: len(si.on_wait) - limit]
                    si.on_wait = si.on_wait[len(si.on_wait) - limit :]
                    for k in range(0, len(extra), limit):
                        new.append(fresh_nop(inst.engine, extra[k : k + limit]))
                    changed = True
                new.append(inst)
            if changed:
                bb.instructions = new


def build_bass(n_groups=N_GROUPS, reps=1):
    """Build the Bass module for one core processing n_groups row-groups.
    reps>1 repeats the whole pipeline in-NEFF (for timing calibration)."""
    _patch_tile_drain()
    from concourse import bass, mybir, tile

    f32 = mybir.dt.float32
    nc = bass.Bass("TRN2", target_bir_lowering=False, debug=False)
    xin = nc.dram_tensor("xin", [n_groups, 128, 4096], f32, kind="ExternalInput")
    wts = nc.dram_tensor("wts", [128, 16, 128], f32, kind="ExternalInput")
    trig = nc.dram_tensor("trig", [128, 24], f32, kind="ExternalInput")
    yout = nc.dram_tensor("yout", [n_groups, 128, 4096], f32, kind="ExternalOutput")

    mult = mybir.AluOpType.mult
    add = mybir.AluOpType.add
    copy_fn = mybir.ActivationFunctionType.Copy

    with tile.TileContext(nc) as tc:
        with (
            tc.tile_pool(name="wp", bufs=1) as wp,
            tc.tile_pool(name="xp", bufs=3) as xp,
            tc.tile_pool(name="yp", bufs=3) as yp,
            tc.tile_pool(name="sp", bufs=3) as stp,
            tc.tile_pool(name="tp", bufs=6) as tp,
            tc.tile_pool(name="ps", bufs=8, space="PSUM") as psp,
        ):
            wt = wp.tile([128, 16, 128], f32)
            nc.sync.dma_start(wt[:], wts.ap()[:])
            tg = wp.tile([128, 24], f32)
            nc.sync.dma_start(tg[:], trig.ap()[:])

            for g in [g for _ in range(reps) for g in range(n_groups)]:
                xt = xp.tile([128, 4096], f32)
                nc.sync.dma_start(xt[:, 0:2048], xin.ap()[g][:, 0:2048])
                nc.sync.dma_start(xt[:, 2048:4096], xin.ap()[g][:, 2048:4096])
                yt = yp.tile([128, 4096], f32)
                st = stp.tile([128, 4096], f32)
                # per half: 8 matmuls (stages 0..7), then stage 8 in-half
                for h in range(2):
                    ps = []
                    for lc in range(4):
                        c = h * 4 + lc
                        p = psp.tile([128, 512], f32, tag="ps")
                        for t in range(2):
                            ci = 2 * (c // 2) + t
                            nc.tensor.matmul(
                                p[:],
                                wt[:, c * 2 + t, :],
                                xt[:, ci * 512 : (ci + 1) * 512],
                                start=(t == 0),
                                stop=(t == 1),
                            )
                        ps.append(p)
                    # stage 8 pairs within this half: (a, b) = (h*4, h*4+2), (h*4+1, h*4+3)
                    for k in range(2):
                        a, b = h * 4 + k, h * 4 + k + 2
                        pidx = h * 2 + k
                        pa, pb = ps[k], ps[k + 2]
                        t1 = tp.tile([128, 512], f32, tag="t")
                        nc.scalar.activation(
                            t1[:], pb[:], copy_fn, scale=tg[:, 8 + pidx : 9 + pidx]
                        )
                        nc.vector.scalar_tensor_tensor(
                            st[:, a * 512 : (a + 1) * 512],
                            pa[:], tg[:, pidx : pidx + 1], t1[:], mult, add,
                        )
                        t2 = tp.tile([128, 512], f32, tag="t")
                        nc.scalar.activation(
                            t2[:], pb[:], copy_fn, scale=tg[:, pidx : pidx + 1]
                        )
                        nc.vector.scalar_tensor_tensor(
                            st[:, b * 512 : (b + 1) * 512],
                            pa[:], tg[:, 4 + pidx : 5 + pidx], t2[:], mult, add,
                        )
                # stage 9 pairs across halves: (cg, cg+4)
                for cg in range(4):
                    sa = st[:, cg * 512 : (cg + 1) * 512]
                    sb = st[:, (cg + 4) * 512 : (cg + 5) * 512]
                    t3 = tp.tile([128, 512], f32, tag="t")
                    nc.scalar.activation(
                        t3[:], sb[:], copy_fn, scale=tg[:, 20 + cg : 21 + cg]
                    )
                    nc.vector.scalar_tensor_tensor(
                        yt[:, cg * 1024 : cg * 1024 + 512],
                        sa, tg[:, 12 + cg : 13 + cg], t3[:], mult, add,
                    )
                    t4 = tp.tile([128, 512], f32, tag="t")
                    nc.scalar.activation(
                        t4[:], sb[:], copy_fn, scale=tg[:, 12 + cg : 13 + cg]
                    )
                    nc.vector.scalar_tensor_tensor(
                        yt[:, cg * 1024 + 512 : (cg + 1) * 1024],
                        sa, tg[:, 16 + cg : 17 + cg], t4[:], mult, add,
                    )
                    nc.scalar.dma_start(
                        yout.ap()[g][:, cg * 1024 : (cg + 1) * 1024],
                        yt[:, cg * 1024 : (cg + 1) * 1024],
                    )
    _split_multi_waits(nc)
    return nc


_CACHE = {}


def _get_nc(n_groups=N_GROUPS):
    if n_groups not in _CACHE:
        _CACHE[n_groups] = build_bass(n_groups)
    return _CACHE[n_groups]


def make_in_maps(x, angles):
    """Pack full inputs into per-core in_maps (list of dicts)."""
    x = np.asarray(x, dtype=np.float32)
    angles = np.asarray(angles, dtype=np.float32)
    wts, trig = _host_tables(angles)
    flat = x.reshape(-1, DIM)
    in_maps = []
    for k in range(N_CORES):
        shard = flat[k * ROWS_PER_CORE : (k + 1) * ROWS_PER_CORE]
        in_maps.append({"xin": _pack_x(shard), "wts": wts, "trig": trig})
    return in_maps


def kernel(x, angles):
    from concourse.bass_utils import run_bass_kernel_spmd

    x = np.asarray(x)
    orig_shape = x.shape
    in_maps = make_in_maps(x, angles)
    nc = _get_nc()
    res = run_bass_kernel_spmd(nc, in_maps, core_ids=list(range(N_CORES)))
    parts = [_unpack_y(res.results[k]["yout"]) for k in range(N_CORES)]
    out = np.concatenate(parts, axis=0).reshape(orig_shape)
    return out.astype(np.float32)
